# revision 26
# baseline (speedup 1.0000x reference)
"""Trainium2 Bass kernel for nn_BoxEncoder (B=128, T=200, NC=3, NB=2, D=512, DH=256).

Data-parallel over batch: 16 batch items per core x 8 cores. Token layout per
core: partition p = bt*8 + q (bt = batch item 0..15, q = 0..7). Each partition
owns 225 j-slots: j in [0,75) dist tokens, j in [75,225) box tokens.

v2 pipeline (vs v1):
 - LayerNorm mean folded into W1 on the host (W1c = W1 - rowmean), so
   z = x @ W1c is already centered; no bias term in the LN.
 - LN variance via the Gram trick: var = x^T G x / 256 with G = W1c W1c^T
   precomputed on host. Computed per tile with one tiny K=32/N=32 matmul
   plus one DVE tensor_tensor_reduce, so z is computed exactly ONCE
   (no separate stats pass over z).
 - rstd for all 150 tiles batched: one ACT Sqrt + one DVE reciprocal
   (exactly two ACT table loads in the whole kernel: Sqrt set, Gelu set).
 - gelu((z)*rstd) applied straight from PSUM with the per-partition scale
   AP; h written to SBUF bf16.
 - h transposed with dma_start_transpose (SBUF->SBUF, 4 tiles per issue)
   instead of PE transposes + PSUM round-trip copies.
 - W2 accumulation (hi/lo/extras) skewed across 3 tiles so no two
   same-bank matmuls are adjacent (avoids the ~380ns PSUM accumulation
   drain stall; independent matmuls issue at full rate).
 - dist tiles (rank-1) run in the Gram phase as PE fillers.
 - Output staged and DMA'd as bf16 (halves HBM write traffic); host
   converts back to f32. Adds ~0.2% relative error, well within budget.
"""

import os
import numpy as np
import ml_dtypes

DBG_PETR = os.environ.get("DBG_PETR", "0") == "1"      # PE transposes instead of DMA transpose
DBG_F32OUT = os.environ.get("DBG_F32OUT", "0") == "1"  # f32 output dram
DBG_NOSKEW = os.environ.get("DBG_NOSKEW", "0") == "1"  # consecutive accumulation groups
DBG_NOTTR = os.environ.get("DBG_NOTTR", "0") == "1"    # tensor_tensor + tensor_reduce instead of ttr
DBG_NOSTATS = os.environ.get("DBG_NOSTATS", "0") == "1"  # stub variance (memset v)

B, T, NCAM, NB, D, DH = 128, 200, 3, 2, 512, 256
IW, IH = 640.0, 400.0
NCORES = 8
BPC = B // NCORES            # batch items per core
JD, JB = 75, 150             # dist / box j-slots per partition
J = JD + JB                  # 225
F = 32                       # feature columns per j-slot
NCHUNK = (J * F + 127) // 128   # 57 transpose chunks (56 full + 1 of 32 cols)
NG = JB // 3                 # 50 box groups of 3 tiles

_CACHE = {}


def _build_nc():
    from contextlib import ExitStack
    import concourse.bacc as bacc
    import concourse.mybir as mybir
    import concourse.tile as tile

    f32 = mybir.dt.float32
    bf16 = mybir.dt.bfloat16
    A = mybir.AluOpType
    AF = mybir.ActivationFunctionType

    # bf16 pack column offsets
    C_W1 = 0
    C_W2HI = C_W1 + 256
    C_W2LO = C_W2HI + 512
    C_W2X = C_W2LO + 512          # 3 cam variants, 512 each
    C_G = C_W2X + 3 * 512
    C_ID = C_G + 128
    NBF = C_ID + 128

    nc = bacc.Bacc("TRN2", target_bir_lowering=False, debug=False,
                   num_devices=NCORES)
    stg_dt = f32 if DBG_F32OUT else bf16
    fpk = nc.declare_dram_parameter("fpk", [128, 900 + 128], f32, isOutput=False)
    bpk = nc.declare_dram_parameter("bpk", [128, NBF], bf16, isOutput=False)
    out_d = nc.declare_dram_parameter("out", [BPC, 1800, D],
                                  f32 if DBG_F32OUT else bf16, isOutput=True)

    with ExitStack() as ctx:
        tc = ctx.enter_context(tile.TileContext(nc))
        cp = ctx.enter_context(tc.tile_pool(name="const", bufs=1))
        sc = ctx.enter_context(tc.tile_pool(name="scratch", bufs=1))
        # PSUM pools (8 banks):  zg 2x2 + op 3x1 + ctp/yp slack
        zgp = ctx.enter_context(tc.tile_pool(name="zgp", bufs=4, space="PSUM"))
        opa = ctx.enter_context(tc.tile_pool(name="opa", bufs=4, space="PSUM"))
        tmpp = ctx.enter_context(tc.tile_pool(name="tmpp", bufs=2))
        hgp = ctx.enter_context(tc.tile_pool(name="hgp", bufs=3))
        htp = ctx.enter_context(tc.tile_pool(name="htp", bufs=5))
        wstp = ctx.enter_context(tc.tile_pool(name="wstp", bufs=2))
        bstg = ctx.enter_context(tc.tile_pool(name="bstage", bufs=3))
        dstg = ctx.enter_context(tc.tile_pool(name="dstage", bufs=3))

        fpack = cp.tile([128, 900 + 128], f32)
        nc.sync.dma_start(fpack[:], fpk[:])
        bpack = cp.tile([128, NBF], bf16)
        nc.sync.dma_start(bpack[:], bpk[:])

        raw = fpack[:, 0:900]
        idf = fpack[:, 900:1028]
        w1 = bpack[:, C_W1:C_W1 + 256]
        w2hi = bpack[:, C_W2HI:C_W2HI + 512]
        w2lo = bpack[:, C_W2LO:C_W2LO + 512]
        w2x = [bpack[:, C_W2X + c * 512: C_W2X + (c + 1) * 512] for c in range(3)]
        Gblk = bpack[:, C_G:C_G + 128]

        TF = cp.tile([128, J * F], f32)
        nc.gpsimd.memset(TF[:], 0.0)

        TFj = TF.rearrange("p (j f) -> p j f", f=F)
        TFd = TFj[:, :JD, :]                       # dist slots
        TFb = TFj[:, JD:, :]                       # box slots
        TFbp = TF[:, JD * F:].rearrange("p (m g f) -> p m g f", g=2, f=F)
        raw6 = raw.rearrange("p (b s) -> p b s", s=6)
        rawp = raw.rearrange("p (m g s) -> p m g s", g=2, s=6)

        # ---------------- P1: feature planes ----------------
        sPres = sc.tile([128, JB], f32)
        sKey = sc.tile([128, JB], f32)
        sSwap = sc.tile([128, JD], f32)
        sD = sc.tile([128, JD], f32)
        sSD = sc.tile([128, JD], f32)
        sw = [sc.tile([128, JB], f32, tag=f"swp{i}", name=f"swp{i}")
              for i in range(6)]
        sT0 = sc.tile([128, JB], f32)
        sT1 = sc.tile([128, JB], f32)

        nc.vector.tensor_tensor(sT0[:], raw6[:, :, 0], raw6[:, :, 1], A.add)
        nc.vector.tensor_tensor(sT1[:], raw6[:, :, 2], raw6[:, :, 3], A.add)
        nc.vector.tensor_tensor(sT0[:], sT0[:], sT1[:], A.add)
        nc.vector.tensor_scalar(sPres[:], sT0[:], 0.0, None, A.not_equal)
        # key = cat - 1000*pres  (order-equivalent to cat + 1000*(1-pres))
        nc.vector.scalar_tensor_tensor(sKey[:], sPres[:], -1000.0,
                                       raw6[:, :, 4], A.mult, A.add)
        sKeyp = sKey.rearrange("p (m g) -> p m g", g=2)
        nc.vector.tensor_tensor(sSwap[:], sKeyp[:, :, 1], sKeyp[:, :, 0], A.is_lt)

        # compare-and-swap each of the 6 raw components + presence
        for i in range(6):
            ve, vo = rawp[:, :, 0, i], rawp[:, :, 1, i]
            dst = sw[i].rearrange("p (m g) -> p m g", g=2)
            nc.vector.tensor_tensor(sD[:], vo, ve, A.subtract)
            nc.vector.tensor_tensor(sSD[:], sD[:], sSwap[:], A.mult)
            nc.vector.tensor_tensor(dst[:, :, 0], ve, sSD[:], A.add)
            nc.vector.tensor_tensor(dst[:, :, 1], vo, sSD[:], A.subtract)
        sPresP = sPres.rearrange("p (m g) -> p m g", g=2)
        nc.vector.tensor_tensor(sD[:], sPresP[:, :, 1], sPresP[:, :, 0], A.subtract)
        nc.vector.tensor_tensor(sSD[:], sD[:], sSwap[:], A.mult)
        nc.vector.tensor_tensor(TFbp[:, :, 0, 14], sPresP[:, :, 0], sSD[:], A.add)
        nc.vector.tensor_tensor(TFbp[:, :, 1, 14], sPresP[:, :, 1], sSD[:], A.subtract)

        sX1, sY1, sX2, sY2, sCat, sConf = sw
        # f0..f3: normalized coords
        nc.vector.tensor_scalar(TFb[:, :, 0], sX1[:], 1.0 / IW, None, A.mult)
        nc.vector.tensor_scalar(TFb[:, :, 1], sY1[:], 1.0 / IH, None, A.mult)
        nc.vector.tensor_scalar(TFb[:, :, 2], sX2[:], 1.0 / IW, None, A.mult)
        nc.vector.tensor_scalar(TFb[:, :, 3], sY2[:], 1.0 / IH, None, A.mult)
        # f4 w, f5 h, f6 cx*2, f7 cy*2 (the 0.5 is folded into the weights)
        nc.vector.tensor_tensor(TFb[:, :, 4], TFb[:, :, 2], TFb[:, :, 0], A.subtract)
        nc.vector.tensor_tensor(TFb[:, :, 5], TFb[:, :, 3], TFb[:, :, 1], A.subtract)
        nc.vector.tensor_tensor(TFb[:, :, 6], TFb[:, :, 0], TFb[:, :, 2], A.add)
        nc.vector.tensor_tensor(TFb[:, :, 7], TFb[:, :, 1], TFb[:, :, 3], A.add)
        # f8 area, f9 aspect = w / (h + 1e-6)
        nc.vector.tensor_tensor(TFb[:, :, 8], TFb[:, :, 4], TFb[:, :, 5], A.mult)
        sHp = sT0
        nc.vector.tensor_scalar(sHp[:], TFb[:, :, 5], 1e-6, None, A.add)
        sR = sT1
        nc.vector.reciprocal(sR[:], sHp[:])
        nc.vector.tensor_tensor(TFb[:, :, 9], TFb[:, :, 4], sR[:], A.mult)
        # f10..12 cat one-hots * pres ; f13 conf*pres ; f15 = 1-pres
        for k in range(3):
            nc.vector.scalar_tensor_tensor(TFb[:, :, 10 + k], sCat[:], float(k),
                                           TFb[:, :, 14], A.is_equal, A.mult)
        nc.vector.tensor_tensor(TFb[:, :, 13], sConf[:], TFb[:, :, 14], A.mult)
        nc.vector.tensor_scalar(TFb[:, :, 15], TFb[:, :, 14], -1.0, 1.0,
                                A.mult, A.add)
        # dist tokens: f16 = 0.5*sqrt(dx2^2+dy2^2) (cx stored doubled), f17 = 1
        sDx = sc.tile([128, JD], f32)
        sDy = sc.tile([128, JD], f32)
        nc.vector.tensor_tensor(sDx[:], TFbp[:, :, 0, 6], TFbp[:, :, 1, 6], A.subtract)
        nc.vector.tensor_tensor(sDy[:], TFbp[:, :, 0, 7], TFbp[:, :, 1, 7], A.subtract)
        nc.vector.tensor_tensor(sDx[:], sDx[:], sDx[:], A.mult)
        nc.vector.tensor_tensor(sDy[:], sDy[:], sDy[:], A.mult)
        nc.vector.tensor_tensor(sDx[:], sDx[:], sDy[:], A.add)
        nc.scalar.activation(TFd[:, :, 16], sDx[:], AF.Sqrt, scale=0.25)
        nc.vector.memset(TFd[:, :, 17], 1.0)

        # ---------------- P2: transpose T_feat chunks -> bf16 lhsT tiles ----
        cta = cp.tile([128, NCHUNK * 128], bf16)
        # garbage rows of the last (short) chunk hit zero blocks of Gblk, but
        # must at least be finite: zero them once
        nc.vector.memset(cta[:, (NCHUNK - 1) * 128:], 0.0)
        for ci in range(NCHUNK):
            w_cols = min(128, J * F - ci * 128)
            ps = opa.tile([128, D], f32, tag="oa", name="oa")[:, 0:128]
            nc.tensor.transpose(ps[:w_cols, :], TF[:, ci * 128: ci * 128 + w_cols],
                                idf)
            dst = cta[:w_cols, ci * 128: ci * 128 + 128]
            if ci % 2 == 0:
                nc.vector.tensor_copy(dst, ps[:w_cols, :])
            else:
                nc.scalar.copy(dst, ps[:w_cols, :])

        def lhsT(j):
            ci, jj = j // 4, j % 4
            return cta[32 * jj: 32 * jj + 32, ci * 128: (ci + 1) * 128]

        # ---------------- P3: Gram variance + dist tiles ----------------
        v = sc.tile([128, JB], f32)

        dist_copy_idx = 0
        dist_stage = None
        vd = out_d[:, 0:600, :].rearrange("b (q r) d -> b q r d", q=8)

        eps = sc.tile([128, 1], f32)
        nc.vector.memset(eps[:], 1e-5)
        sd = sc.tile([128, JB], f32)
        rstd = sc.tile([128, JB], f32)

        def emit_rstd(k0, k1):
            nc.scalar.activation(sd[:, k0:k1], v[:, k0:k1], AF.Sqrt,
                                 bias=eps[:], scale=1.0 / DH)
            nc.vector.reciprocal(rstd[:, k0:k1], sd[:, k0:k1])

        def emit_dist_tile(jd):
            jjd = jd % 4
            o = opa.tile([128, D], f32, tag="oa", name="oa")
            nc.tensor.matmul(o[:], lhsT(jd), w2x[0][32 * jjd: 32 * jjd + 32, :],
                             start=True, stop=True,
                             tile_position=(32 * jjd, 0))
            return o

        for ci in range(JD // 4, NCHUNK):
            y4t = opa.tile([128, D], f32, tag="oa", name="oa")
            y4 = y4t[:, 0:128]
            nc.tensor.matmul(y4, cta[:, ci * 128:(ci + 1) * 128], Gblk,
                             start=True, stop=True)
            j0, j1 = max(4 * ci, JD), min(4 * ci + 4, J)
            if j1 - j0 == 4:
                tmp = tmpp.tile([128, 128], f32, tag="tmp", name="tmp")
                nc.vector.tensor_tensor(tmp[:], TF[:, 4 * ci * F:(4 * ci + 4) * F],
                                        y4[:], A.mult)
                nc.vector.tensor_reduce(v[:, j0 - JD:j1 - JD],
                                        tmp.rearrange("p (j f) -> p j f", f=F),
                                        mybir.AxisListType.X, A.add)
            else:
                for j in range(j0, j1):
                    k = j - JD
                    waste = wstp.tile([128, 32], f32, tag="wst", name="waste")
                    nc.vector.tensor_tensor(waste[:], TFj[:, j, :],
                                            y4[:, 32 * (j % 4):32 * (j % 4) + 32],
                                            A.mult)
                    nc.vector.tensor_reduce(v[:, k:k + 1], waste[:],
                                            mybir.AxisListType.X, A.add)
            # interleave 2 dist tiles per chunk
            for _ in range(2):
                if dist_copy_idx >= JD:
                    continue
                jd = dist_copy_idx
                o = emit_dist_tile(jd)
                if dist_stage is None:
                    dist_stage = dstg.tile([128, 5 * D], stg_dt, tag="dstage",
                                           name="dist_stage")
                slot = jd % 5
                if jd % 3 == 2:
                    nc.vector.tensor_copy(dist_stage[:, slot * D:(slot + 1) * D], o[:])
                else:
                    nc.scalar.copy(dist_stage[:, slot * D:(slot + 1) * D], o[:])
                dist_copy_idx += 1
                if slot == 4:
                    nc.scalar.dma_start(vd[:, :, jd - 4:jd + 1, :], dist_stage[:])
                    dist_stage = None
            if ci == 37:
                emit_rstd(0, 75)
        # leftover dist tiles (39 chunks x 2 = 78 >= 75, none left normally)
        while dist_copy_idx < JD:
            jd = dist_copy_idx
            o = emit_dist_tile(jd)
            if dist_stage is None:
                dist_stage = dstg.tile([128, 5 * D], stg_dt, tag="dstage",
                                       name="dist_stage")
            slot = jd % 5
            nc.scalar.copy(dist_stage[:, slot * D:(slot + 1) * D], o[:])
            dist_copy_idx += 1
            if slot == 4:
                nc.scalar.dma_start(vd[:, :, jd - 4:jd + 1, :], dist_stage[:])
                dist_stage = None

        # ---------------- P3b: second rstd batch ----------------
        emit_rstd(75, JB)

        # ---------------- P4: box pipeline (groups of 3 tiles) ----------------
        # Per tile: accumulation PAIR (w2hi+w2lo) into bank A at full rate,
        # extras as an independent SINGLE into bank B, combined by the DVE
        # staging copy (A+B -> bf16). 3-chains run at ~311ns/matmul on hw,
        # pairs and singles at ~216ns.
        # Iteration order keeps the in-order PE queue stall-free:
        #   [gelu g-1 (ACT), transpose g-1 (SP)] [W slots g-2 (PE)] [z g (PE)]
        zg_t, ht_t, oa_t, ob_t = {}, {}, {}, {}
        stage_state = {"tile": None, "fill": 0}
        vb = out_d[:, 600:1800, :].rearrange("b (q r) d -> b q r d", q=8)

        def emit_slot(s_):
            if s_ < JB:                      # hi_s (start)
                oa_t[s_] = opa.tile([128, D], f32, tag="oa", name="oa")
                ht = ht_t[s_ // 3]
                nc.tensor.matmul(oa_t[s_][:], ht[:, 2 * (s_ % 3), :], w2hi,
                                 start=True, stop=False)
            k = s_ - 1
            if 0 <= k < JB:                  # lo_{s-1}
                ht = ht_t[k // 3]
                nc.tensor.matmul(oa_t[k][:], ht[:, 2 * (k % 3) + 1, :], w2lo,
                                 start=False, stop=False)
                if k % 3 == 2:
                    ht_t.pop(k // 3)
            k = s_ - 2
            if 0 <= k < JB:                  # x_{s-2} (stop) + copy
                j = JD + k
                jj = j % 4
                cam = (k % 6) // 2
                ot = oa_t.pop(k)
                nc.tensor.matmul(ot[:], lhsT(j),
                                 w2x[cam][32 * jj:32 * jj + 32, :],
                                 start=False, stop=True,
                                 tile_position=(32 * jj, 0))
                if stage_state["tile"] is None:
                    stage_state["tile"] = bstg.tile([128, 8 * D], stg_dt,
                                                    tag="bstage", name="bstage")
                    stage_state["fill"] = 0
                fill = stage_state["fill"]
                dst = stage_state["tile"][:, fill * D:(fill + 1) * D]
                if k % 2 == 0:
                    nc.vector.tensor_copy(dst, ot[:])
                else:
                    nc.scalar.copy(dst, ot[:])
                stage_state["fill"] = fill + 1
                if stage_state["fill"] == 8 or k == JB - 1:
                    gsz = stage_state["fill"]
                    j0 = k - gsz + 1
                    nc.scalar.dma_start(vb[:, :, j0:j0 + gsz, :],
                                        stage_state["tile"][:, : gsz * D])
                    stage_state["tile"] = None

        for it in range(NG + 4):
            # stage G: gelu for group g1 + transpose issue
            g1 = it - 1
            if 0 <= g1 < NG:
                hg = hgp.tile([128, 3, DH], bf16, tag="hg", name="hg")
                for q in range(3):
                    k = 3 * g1 + q
                    zq = zg_t.pop((g1, q))
                    nc.scalar.activation(hg[:, q, :], zq, AF.Gelu,
                                         scale=rstd[:, k:k + 1])
                ht = htp.tile([128, 6, 128], bf16, tag="ht", name="ht")
                ht_t[g1] = ht
                nc.sync.dma_start_transpose(ht[:], hg[:])
            # stage W: rolling slots for group g2
            g2 = it - 4
            if 0 <= g2 < NG:
                for q in range(3):
                    emit_slot(3 * g2 + q)
            # stage Z: z matmuls for group g0
            g0 = it
            if g0 < NG:
                for q in range(3):
                    zb = zgp.tile([128, DH], f32, tag="z", name="z")
                    zg_t[(g0, q)] = zb[:]
                    k = 3 * g0 + q
                    j = JD + k
                    jj = j % 4
                    nc.tensor.matmul(zb[:], lhsT(j),
                                     w1[32 * jj:32 * jj + 32, :],
                                     start=True, stop=True,
                                     tile_position=(32 * jj, 0))
        emit_slot(JB)
        emit_slot(JB + 1)

    nc.compile()
    return nc


def _prep_inputs(inputs):
    f32 = np.float32
    bf = ml_dtypes.bfloat16
    scale = float(np.asarray(inputs["scale"]))

    W1p = np.zeros((32, DH), f32)
    W1p[0:10] = np.asarray(inputs["geom_w1"], f32)
    W1p[6] *= 0.5
    W1p[7] *= 0.5
    W1p -= W1p.mean(axis=1, keepdims=True)      # fold LN mean into W1
    w1rep = np.tile(W1p, (4, 1))

    G = (W1p @ W1p.T).astype(f32)               # gram for LN variance
    Gblk = np.zeros((128, 128), f32)            # block-diag(G x4)
    for t in range(4):
        Gblk[32 * t:32 * t + 32, 32 * t:32 * t + 32] = G

    W2s = scale * np.asarray(inputs["geom_w2"], f32)
    w2hi, w2lo = W2s[:128], W2s[128:]

    cat_t = np.asarray(inputs["cat_table"], f32)
    cam_t = np.asarray(inputs["cam_table"], f32)
    bias_row = (np.asarray(inputs["geom_b2"], f32)
                + np.asarray(inputs["conf_b"], f32)
                + np.asarray(inputs["center_b"], f32))
    w2x_reps = []
    for c in range(3):
        W2x = np.zeros((32, D), f32)
        W2x[6] = scale * np.asarray(inputs["center_w"], f32)[0] * 0.5
        W2x[7] = scale * np.asarray(inputs["center_w"], f32)[1] * 0.5
        W2x[10:13] = scale * cat_t
        W2x[13] = scale * np.asarray(inputs["conf_w"], f32)[0]
        W2x[14] = scale * (bias_row + cam_t[c])
        W2x[15] = np.asarray(inputs["missing_emb"], f32)[0]
        W2x[16] = np.asarray(inputs["dist_w"], f32)[0]
        W2x[17] = np.asarray(inputs["dist_b"], f32)
        w2x_reps.append(np.tile(W2x, (4, 1)))

    idf32 = np.eye(128, dtype=f32)
    bpk = np.concatenate(
        [w1rep, w2hi, w2lo] + w2x_reps + [Gblk, idf32], axis=1
    ).astype(bf)

    box = np.asarray(inputs["box_data"], f32)
    fpks = []
    for c in range(NCORES):
        rawc = box[c * BPC:(c + 1) * BPC].reshape(BPC, T * 6, 6)
        rawc = rawc.reshape(BPC, 8, JB, 6).reshape(128, 900)
        fpks.append(np.ascontiguousarray(
            np.concatenate([rawc, idf32], axis=1), dtype=f32))
    return fpks, bpk


def _fast_path_ok(inputs):
    try:
        shapes = {
            "box_data": (B, T, 6, 6), "cat_table": (3, D), "geom_w1": (10, DH),
            "geom_b1": (DH,), "ln_g": (DH,), "ln_b": (DH,), "geom_w2": (DH, D),
            "geom_b2": (D,), "conf_w": (1, D), "conf_b": (D,),
            "center_w": (2, D), "center_b": (D,), "missing_emb": (1, D),
            "dist_w": (1, D), "dist_b": (D,), "cam_table": (NCAM, D),
        }
        for k, s in shapes.items():
            if tuple(np.asarray(inputs[k]).shape) != s:
                return False
        if not np.all(np.asarray(inputs["geom_b1"]) == 0):
            return False
        if not np.all(np.asarray(inputs["ln_g"]) == 1):
            return False
        if not np.all(np.asarray(inputs["ln_b"]) == 0):
            return False
        return True
    except Exception:
        return False


def _numpy_fallback(inputs):
    # Exact (slow) port of the reference for unexpected inputs.
    import math
    f32 = np.float32
    inp = {k: np.asarray(v) for k, v in inputs.items()}
    coords = inp["box_data"][..., :4].astype(f32)
    category = inp["box_data"][..., 4].astype(np.int32)
    conf = inp["box_data"][..., 5].astype(f32)
    norm = np.array([IW, IH, IW, IH], f32)
    cn = (coords / norm).reshape(B, T, NCAM, NB, 4)
    category = category.reshape(B, T, NCAM, NB)
    conf = conf.reshape(B, T, NCAM, NB, 1)
    presence = (cn.sum(-1) != 0).astype(f32)
    sort_key = category.astype(f32) + (1.0 - presence) * 1000.0
    idx = np.argsort(sort_key, axis=-1, kind="stable")
    cn = np.take_along_axis(cn, idx[..., None], axis=-2)
    category = np.take_along_axis(category, idx, axis=-1)
    conf = np.take_along_axis(conf, idx[..., None], axis=-2)
    presence = (cn.sum(-1) != 0).astype(f32)[..., None]
    x1, y1, x2, y2 = cn[..., 0], cn[..., 1], cn[..., 2], cn[..., 3]
    w, h = x2 - x1, y2 - y1
    cx, cy = (x1 + x2) * 0.5, (y1 + y2) * 0.5
    area, aspect = w * h, w / (h + 1e-6)
    dx, dy = cx[..., 0] - cx[..., 1], cy[..., 0] - cy[..., 1]
    dist = np.sqrt(dx * dx + dy * dy)[..., None]
    dist_tok = dist @ inp["dist_w"].astype(f32) + inp["dist_b"].astype(f32)
    geom = np.stack([x1, y1, x2, y2, w, h, cx, cy, area, aspect], axis=-1)
    z = geom @ inp["geom_w1"].astype(f32) + inp["geom_b1"].astype(f32)
    mu = z.mean(-1, keepdims=True)
    var = ((z - mu) ** 2).mean(-1, keepdims=True)
    xh = (z - mu) / np.sqrt(var + 1e-5) * inp["ln_g"].astype(f32) + inp["ln_b"].astype(f32)
    try:
        from scipy.special import erf as _erf
        g = xh * 0.5 * (1.0 + _erf(xh / np.sqrt(2.0)))
    except Exception:
        verf = np.vectorize(math.erf)
        g = xh * 0.5 * (1.0 + verf(xh / np.sqrt(2.0)))
    geom_p = g @ inp["geom_w2"].astype(f32) + inp["geom_b2"].astype(f32)
    cat_emb = inp["cat_table"].astype(f32)[category]
    conf_p = conf @ inp["conf_w"].astype(f32) + inp["conf_b"].astype(f32)
    center_p = np.stack([cx, cy], axis=-1) @ inp["center_w"].astype(f32) + inp["center_b"].astype(f32)
    cam_emb = inp["cam_table"].astype(f32).reshape(1, 1, NCAM, 1, D)
    tok = (geom_p + cat_emb + conf_p + center_p + cam_emb) * float(inp["scale"])
    tok = np.where(presence == 0, inp["missing_emb"].astype(f32)[0], tok)
    out = np.concatenate([dist_tok.reshape(B, T * NCAM, D),
                          tok.reshape(B, T * NCAM * NB, D)], axis=1)
    return out.astype(np.float32)


def _run(inputs, trace=False, tmpdir=None):
    from concourse.bass_utils import run_bass_kernel_spmd

    if "nc" not in _CACHE:
        _CACHE["nc"] = _build_nc()
    nc = _CACHE["nc"]

    fpks, bpk = _prep_inputs(inputs)
    in_maps = [{"fpk": fpks[c], "bpk": bpk} for c in range(NCORES)]
    res = run_bass_kernel_spmd(nc, in_maps, list(range(NCORES)),
                               trace=trace, tmpdir=tmpdir)
    out = np.concatenate([np.asarray(res.results[c]["out"])
                          for c in range(NCORES)], axis=0)
    return out.astype(np.float32), res


def kernel(**inputs):
    if not _fast_path_ok(inputs):
        return _numpy_fallback(inputs)
    out, _ = _run(inputs)
    return out


if __name__ == "__main__":
    import reference as ref
    inputs = {k: np.asarray(v) for k, v in ref.setup_inputs().items()}
    got = kernel(**inputs)
    exp = np.load("/tmp/expected.npy")
    d = got - exp
    print("rel fro:", np.linalg.norm(d) / np.linalg.norm(exp))
    print("absmax rel:", np.abs(d).max() / np.abs(exp).max())


# revision 27
# speedup vs baseline: 1.0690x; 1.0690x over previous
"""Trainium2 Bass kernel for nn_BoxEncoder (B=128, T=200, NC=3, NB=2, D=512, DH=256).

Data-parallel over batch: 16 batch items per core x 8 cores. Token layout per
core: partition p = bt*8 + q (bt = batch item 0..15, q = 0..7). Each partition
owns 225 j-slots: j in [0,75) dist tokens, j in [75,225) box tokens.

v2 pipeline (vs v1):
 - LayerNorm mean folded into W1 on the host (W1c = W1 - rowmean), so
   z = x @ W1c is already centered; no bias term in the LN.
 - LN variance via the Gram trick: var = x^T G x / 256 with G = W1c W1c^T
   precomputed on host. Computed per tile with one tiny K=32/N=32 matmul
   plus one DVE tensor_tensor_reduce, so z is computed exactly ONCE
   (no separate stats pass over z).
 - rstd for all 150 tiles batched: one ACT Sqrt + one DVE reciprocal
   (exactly two ACT table loads in the whole kernel: Sqrt set, Gelu set).
 - gelu((z)*rstd) applied straight from PSUM with the per-partition scale
   AP; h written to SBUF bf16.
 - h transposed with dma_start_transpose (SBUF->SBUF, 4 tiles per issue)
   instead of PE transposes + PSUM round-trip copies.
 - W2 accumulation (hi/lo/extras) skewed across 3 tiles so no two
   same-bank matmuls are adjacent (avoids the ~380ns PSUM accumulation
   drain stall; independent matmuls issue at full rate).
 - dist tiles (rank-1) run in the Gram phase as PE fillers.
 - Output staged and DMA'd as bf16 (halves HBM write traffic); host
   converts back to f32. Adds ~0.2% relative error, well within budget.
"""

import os
import numpy as np
import ml_dtypes

DBG_PETR = os.environ.get("DBG_PETR", "0") == "1"      # PE transposes instead of DMA transpose
DBG_F32OUT = os.environ.get("DBG_F32OUT", "0") == "1"  # f32 output dram
DBG_NOSKEW = os.environ.get("DBG_NOSKEW", "0") == "1"  # consecutive accumulation groups
DBG_NOTTR = os.environ.get("DBG_NOTTR", "0") == "1"    # tensor_tensor + tensor_reduce instead of ttr
DBG_NOSTATS = os.environ.get("DBG_NOSTATS", "0") == "1"  # stub variance (memset v)

B, T, NCAM, NB, D, DH = 128, 200, 3, 2, 512, 256
IW, IH = 640.0, 400.0
NCORES = 8
BPC = B // NCORES            # batch items per core
JD, JB = 75, 150             # dist / box j-slots per partition
J = JD + JB                  # 225
F = 32                       # feature columns per j-slot
NCHUNK = (J * F + 127) // 128   # 57 transpose chunks (56 full + 1 of 32 cols)
NG = JB // 3                 # 50 box groups of 3 tiles

_CACHE = {}


def _build_nc():
    from contextlib import ExitStack
    import concourse.bacc as bacc
    import concourse.mybir as mybir
    import concourse.tile as tile

    f32 = mybir.dt.float32
    bf16 = mybir.dt.bfloat16
    A = mybir.AluOpType
    AF = mybir.ActivationFunctionType

    # bf16 pack column offsets
    C_W1 = 0
    C_W2HI = C_W1 + 256
    C_W2LO = C_W2HI + 512
    C_W2X = C_W2LO + 512          # 3 cam variants, 512 each
    C_G = C_W2X + 3 * 512
    C_ID = C_G + 128
    NBF = C_ID + 128

    nc = bacc.Bacc("TRN2", target_bir_lowering=False, debug=False,
                   num_devices=NCORES)
    stg_dt = f32 if DBG_F32OUT else bf16
    fpk = nc.declare_dram_parameter("fpk", [128, 900 + 128], f32, isOutput=False)
    bpk = nc.declare_dram_parameter("bpk", [128, NBF], bf16, isOutput=False)
    out_d = nc.declare_dram_parameter("out", [BPC, 1800, D],
                                  f32 if DBG_F32OUT else bf16, isOutput=True)

    with ExitStack() as ctx:
        tc = ctx.enter_context(tile.TileContext(nc))
        cp = ctx.enter_context(tc.tile_pool(name="const", bufs=1))
        sc = ctx.enter_context(tc.tile_pool(name="scratch", bufs=1))
        # PSUM pools (8 banks):  zg 2x2 + op 3x1 + ctp/yp slack
        zgp = ctx.enter_context(tc.tile_pool(name="zgp", bufs=4, space="PSUM"))
        opa = ctx.enter_context(tc.tile_pool(name="opa", bufs=4, space="PSUM"))
        tmpp = ctx.enter_context(tc.tile_pool(name="tmpp", bufs=2))
        hgp = ctx.enter_context(tc.tile_pool(name="hgp", bufs=3))
        htp = ctx.enter_context(tc.tile_pool(name="htp", bufs=5))
        wstp = ctx.enter_context(tc.tile_pool(name="wstp", bufs=2))
        bstg = ctx.enter_context(tc.tile_pool(name="bstage", bufs=3))
        dstg = ctx.enter_context(tc.tile_pool(name="dstage", bufs=3))

        fpack = cp.tile([128, 900 + 128], f32)
        nc.sync.dma_start(fpack[:], fpk[:])
        bpack = cp.tile([128, NBF], bf16)
        nc.sync.dma_start(bpack[:], bpk[:])

        raw = fpack[:, 0:900]
        idf = fpack[:, 900:1028]
        w1 = bpack[:, C_W1:C_W1 + 256]
        w2hi = bpack[:, C_W2HI:C_W2HI + 512]
        w2lo = bpack[:, C_W2LO:C_W2LO + 512]
        w2x = [bpack[:, C_W2X + c * 512: C_W2X + (c + 1) * 512] for c in range(3)]
        Gblk = bpack[:, C_G:C_G + 128]

        TF = cp.tile([128, J * F], f32)
        nc.gpsimd.memset(TF[:], 0.0)

        TFj = TF.rearrange("p (j f) -> p j f", f=F)
        TFd = TFj[:, :JD, :]                       # dist slots
        TFb = TFj[:, JD:, :]                       # box slots
        TFbp = TF[:, JD * F:].rearrange("p (m g f) -> p m g f", g=2, f=F)
        raw6 = raw.rearrange("p (b s) -> p b s", s=6)
        rawp = raw.rearrange("p (m g s) -> p m g s", g=2, s=6)

        # ---------------- P1: feature planes ----------------
        sPres = sc.tile([128, JB], f32)
        sKey = sc.tile([128, JB], f32)
        sSwap = sc.tile([128, JD], f32)
        sD = sc.tile([128, JD], f32)
        sSD = sc.tile([128, JD], f32)
        sw = [sc.tile([128, JB], f32, tag=f"swp{i}", name=f"swp{i}")
              for i in range(6)]
        sT0 = sc.tile([128, JB], f32)
        sT1 = sc.tile([128, JB], f32)

        nc.vector.tensor_tensor(sT0[:], raw6[:, :, 0], raw6[:, :, 1], A.add)
        nc.vector.tensor_tensor(sT1[:], raw6[:, :, 2], raw6[:, :, 3], A.add)
        nc.vector.tensor_tensor(sT0[:], sT0[:], sT1[:], A.add)
        nc.vector.tensor_scalar(sPres[:], sT0[:], 0.0, None, A.not_equal)
        # key = cat - 1000*pres  (order-equivalent to cat + 1000*(1-pres))
        nc.vector.scalar_tensor_tensor(sKey[:], sPres[:], -1000.0,
                                       raw6[:, :, 4], A.mult, A.add)
        sKeyp = sKey.rearrange("p (m g) -> p m g", g=2)
        nc.vector.tensor_tensor(sSwap[:], sKeyp[:, :, 1], sKeyp[:, :, 0], A.is_lt)

        # compare-and-swap each of the 6 raw components + presence
        for i in range(6):
            ve, vo = rawp[:, :, 0, i], rawp[:, :, 1, i]
            dst = sw[i].rearrange("p (m g) -> p m g", g=2)
            nc.vector.tensor_tensor(sD[:], vo, ve, A.subtract)
            nc.vector.tensor_tensor(sSD[:], sD[:], sSwap[:], A.mult)
            nc.vector.tensor_tensor(dst[:, :, 0], ve, sSD[:], A.add)
            nc.vector.tensor_tensor(dst[:, :, 1], vo, sSD[:], A.subtract)
        sPresP = sPres.rearrange("p (m g) -> p m g", g=2)
        nc.vector.tensor_tensor(sD[:], sPresP[:, :, 1], sPresP[:, :, 0], A.subtract)
        nc.vector.tensor_tensor(sSD[:], sD[:], sSwap[:], A.mult)
        nc.vector.tensor_tensor(TFbp[:, :, 0, 14], sPresP[:, :, 0], sSD[:], A.add)
        nc.vector.tensor_tensor(TFbp[:, :, 1, 14], sPresP[:, :, 1], sSD[:], A.subtract)

        sX1, sY1, sX2, sY2, sCat, sConf = sw
        # f0..f3: normalized coords
        nc.vector.tensor_scalar(TFb[:, :, 0], sX1[:], 1.0 / IW, None, A.mult)
        nc.vector.tensor_scalar(TFb[:, :, 1], sY1[:], 1.0 / IH, None, A.mult)
        nc.vector.tensor_scalar(TFb[:, :, 2], sX2[:], 1.0 / IW, None, A.mult)
        nc.vector.tensor_scalar(TFb[:, :, 3], sY2[:], 1.0 / IH, None, A.mult)
        # f4 w, f5 h, f6 cx*2, f7 cy*2 (the 0.5 is folded into the weights)
        nc.vector.tensor_tensor(TFb[:, :, 4], TFb[:, :, 2], TFb[:, :, 0], A.subtract)
        nc.vector.tensor_tensor(TFb[:, :, 5], TFb[:, :, 3], TFb[:, :, 1], A.subtract)
        nc.vector.tensor_tensor(TFb[:, :, 6], TFb[:, :, 0], TFb[:, :, 2], A.add)
        nc.vector.tensor_tensor(TFb[:, :, 7], TFb[:, :, 1], TFb[:, :, 3], A.add)
        # f8 area, f9 aspect = w / (h + 1e-6)
        nc.vector.tensor_tensor(TFb[:, :, 8], TFb[:, :, 4], TFb[:, :, 5], A.mult)
        sHp = sT0
        nc.vector.tensor_scalar(sHp[:], TFb[:, :, 5], 1e-6, None, A.add)
        sR = sT1
        nc.vector.reciprocal(sR[:], sHp[:])
        nc.vector.tensor_tensor(TFb[:, :, 9], TFb[:, :, 4], sR[:], A.mult)
        # f10..12 cat one-hots * pres ; f13 conf*pres ; f15 = 1-pres
        for k in range(3):
            nc.vector.scalar_tensor_tensor(TFb[:, :, 10 + k], sCat[:], float(k),
                                           TFb[:, :, 14], A.is_equal, A.mult)
        nc.vector.tensor_tensor(TFb[:, :, 13], sConf[:], TFb[:, :, 14], A.mult)
        nc.vector.tensor_scalar(TFb[:, :, 15], TFb[:, :, 14], -1.0, 1.0,
                                A.mult, A.add)
        # dist tokens: f16 = 0.5*sqrt(dx2^2+dy2^2) (cx stored doubled), f17 = 1
        sDx = sc.tile([128, JD], f32)
        sDy = sc.tile([128, JD], f32)
        nc.vector.tensor_tensor(sDx[:], TFbp[:, :, 0, 6], TFbp[:, :, 1, 6], A.subtract)
        nc.vector.tensor_tensor(sDy[:], TFbp[:, :, 0, 7], TFbp[:, :, 1, 7], A.subtract)
        nc.vector.tensor_tensor(sDx[:], sDx[:], sDx[:], A.mult)
        nc.vector.tensor_tensor(sDy[:], sDy[:], sDy[:], A.mult)
        nc.vector.tensor_tensor(sDx[:], sDx[:], sDy[:], A.add)
        nc.scalar.activation(TFd[:, :, 16], sDx[:], AF.Sqrt, scale=0.25)
        nc.vector.memset(TFd[:, :, 17], 1.0)

        # ---------------- P2: transpose T_feat chunks -> bf16 lhsT tiles ----
        cta = cp.tile([128, NCHUNK * 128], bf16)
        # garbage rows of the last (short) chunk hit zero blocks of Gblk, but
        # must at least be finite: zero them once
        nc.vector.memset(cta[:, (NCHUNK - 1) * 128:], 0.0)
        for ci in range(NCHUNK):
            w_cols = min(128, J * F - ci * 128)
            ps = opa.tile([128, D], f32, tag="oa", name="oa")[:, 0:128]
            nc.tensor.transpose(ps[:w_cols, :], TF[:, ci * 128: ci * 128 + w_cols],
                                idf)
            dst = cta[:w_cols, ci * 128: ci * 128 + 128]
            if ci % 2 == 0:
                nc.vector.tensor_copy(dst, ps[:w_cols, :])
            else:
                nc.scalar.copy(dst, ps[:w_cols, :])

        def lhsT(j):
            ci, jj = j // 4, j % 4
            return cta[32 * jj: 32 * jj + 32, ci * 128: (ci + 1) * 128]

        # ---------------- P3: Gram variance + dist tiles ----------------
        v = sc.tile([128, JB], f32)

        dist_copy_idx = 0
        dist_stage = None
        vd = out_d[:, 0:600, :].rearrange("b (q r) d -> b q r d", q=8)

        eps = sc.tile([128, 1], f32)
        nc.vector.memset(eps[:], 1e-5)
        sd = sc.tile([128, JB], f32)
        rstd = sc.tile([128, JB], f32)

        def emit_rstd(k0, k1):
            nc.scalar.activation(sd[:, k0:k1], v[:, k0:k1], AF.Sqrt,
                                 bias=eps[:], scale=1.0 / DH)
            nc.vector.reciprocal(rstd[:, k0:k1], sd[:, k0:k1])

        def emit_dist_tile(jd):
            jjd = jd % 4
            o = opa.tile([128, D], f32, tag="oa", name="oa")
            nc.tensor.matmul(o[:], lhsT(jd), w2x[0][32 * jjd: 32 * jjd + 32, :],
                             start=True, stop=True,
                             tile_position=(32 * jjd, 0))
            return o

        for ci in range(JD // 4, NCHUNK):
            y4t = opa.tile([128, D], f32, tag="oa", name="oa")
            y4 = y4t[:, 0:128]
            nc.tensor.matmul(y4, cta[:, ci * 128:(ci + 1) * 128], Gblk,
                             start=True, stop=True)
            j0, j1 = max(4 * ci, JD), min(4 * ci + 4, J)
            if j1 - j0 == 4:
                tmp = tmpp.tile([128, 128], f32, tag="tmp", name="tmp")
                nc.vector.tensor_tensor(tmp[:], TF[:, 4 * ci * F:(4 * ci + 4) * F],
                                        y4[:], A.mult)
                nc.vector.tensor_reduce(v[:, j0 - JD:j1 - JD],
                                        tmp.rearrange("p (j f) -> p j f", f=F),
                                        mybir.AxisListType.X, A.add)
            else:
                for j in range(j0, j1):
                    k = j - JD
                    waste = wstp.tile([128, 32], f32, tag="wst", name="waste")
                    nc.vector.tensor_tensor(waste[:], TFj[:, j, :],
                                            y4[:, 32 * (j % 4):32 * (j % 4) + 32],
                                            A.mult)
                    nc.vector.tensor_reduce(v[:, k:k + 1], waste[:],
                                            mybir.AxisListType.X, A.add)
            # interleave 2 dist tiles per chunk
            for _ in range(2):
                if dist_copy_idx >= JD:
                    continue
                jd = dist_copy_idx
                o = emit_dist_tile(jd)
                if dist_stage is None:
                    dist_stage = dstg.tile([128, 5 * D], stg_dt, tag="dstage",
                                           name="dist_stage")
                slot = jd % 5
                if jd % 3 == 2:
                    nc.vector.tensor_copy(dist_stage[:, slot * D:(slot + 1) * D], o[:])
                else:
                    nc.scalar.copy(dist_stage[:, slot * D:(slot + 1) * D], o[:])
                dist_copy_idx += 1
                if slot == 4:
                    nc.sync.dma_start(vd[:, :, jd - 4:jd + 1, :], dist_stage[:])
                    dist_stage = None
            if ci == 37:
                emit_rstd(0, 75)
        # leftover dist tiles (39 chunks x 2 = 78 >= 75, none left normally)
        while dist_copy_idx < JD:
            jd = dist_copy_idx
            o = emit_dist_tile(jd)
            if dist_stage is None:
                dist_stage = dstg.tile([128, 5 * D], stg_dt, tag="dstage",
                                       name="dist_stage")
            slot = jd % 5
            nc.scalar.copy(dist_stage[:, slot * D:(slot + 1) * D], o[:])
            dist_copy_idx += 1
            if slot == 4:
                nc.sync.dma_start(vd[:, :, jd - 4:jd + 1, :], dist_stage[:])
                dist_stage = None

        # ---------------- P3b: second rstd batch ----------------
        emit_rstd(75, JB)

        # ---------------- P4: box pipeline (groups of 3 tiles) ----------------
        # Per tile: accumulation PAIR (w2hi+w2lo) into bank A at full rate,
        # extras as an independent SINGLE into bank B, combined by the DVE
        # staging copy (A+B -> bf16). 3-chains run at ~311ns/matmul on hw,
        # pairs and singles at ~216ns.
        # Iteration order keeps the in-order PE queue stall-free:
        #   [gelu g-1 (ACT), transpose g-1 (SP)] [W slots g-2 (PE)] [z g (PE)]
        zg_t, ht_t, oa_t, ob_t = {}, {}, {}, {}
        stage_state = {"tile": None, "fill": 0, "pending": None}
        vb = out_d[:, 600:1800, :].rearrange("b (q r) d -> b q r d", q=8)

        def emit_slot(s_):
            if s_ < JB:                      # hi_s (start)
                oa_t[s_] = opa.tile([128, D], f32, tag="oa", name="oa")
                ht = ht_t[s_ // 3]
                nc.tensor.matmul(oa_t[s_][:], ht[:, 2 * (s_ % 3), :], w2hi,
                                 start=True, stop=False)
            k = s_ - 1
            if 0 <= k < JB:                  # lo_{s-1}
                ht = ht_t[k // 3]
                nc.tensor.matmul(oa_t[k][:], ht[:, 2 * (k % 3) + 1, :], w2lo,
                                 start=False, stop=False)
                if k % 3 == 2:
                    ht_t.pop(k // 3)
            k = s_ - 2
            if 0 <= k < JB:                  # x_{s-2} (stop) + copy
                j = JD + k
                jj = j % 4
                cam = (k % 6) // 2
                ot = oa_t.pop(k)
                nc.tensor.matmul(ot[:], lhsT(j),
                                 w2x[cam][32 * jj:32 * jj + 32, :],
                                 start=False, stop=True,
                                 tile_position=(32 * jj, 0))
                if stage_state["tile"] is None:
                    stage_state["tile"] = bstg.tile([128, 8 * D], stg_dt,
                                                    tag="bstage", name="bstage")
                    stage_state["fill"] = 0
                fill = stage_state["fill"]
                dst = stage_state["tile"][:, fill * D:(fill + 1) * D]
                if k % 2 == 0:
                    nc.vector.tensor_copy(dst, ot[:])
                else:
                    nc.scalar.copy(dst, ot[:])
                stage_state["fill"] = fill + 1
                if stage_state["fill"] == 8 or k == JB - 1:
                    gsz = stage_state["fill"]
                    j0 = k - gsz + 1
                    stage_state["pending"] = (stage_state["tile"], j0, gsz)
                    stage_state["tile"] = None

        for it in range(NG + 4):
            # stage G: gelu for group g1 + transpose issue
            g1 = it - 1
            if 0 <= g1 < NG:
                hg = hgp.tile([128, 3, DH], bf16, tag="hg", name="hg")
                for q in range(3):
                    k = 3 * g1 + q
                    zq = zg_t.pop((g1, q))
                    nc.scalar.activation(hg[:, q, :], zq, AF.Gelu,
                                         scale=rstd[:, k:k + 1])
                ht = htp.tile([128, 6, 128], bf16, tag="ht", name="ht")
                ht_t[g1] = ht
                nc.sync.dma_start_transpose(ht[:], hg[:])
            # flush last iteration's staged DMA now that its copies are done
            if stage_state["pending"] is not None:
                ptile, pj0, pgsz = stage_state["pending"]
                nc.sync.dma_start(vb[:, :, pj0:pj0 + pgsz, :], ptile[:, : pgsz * D])
                stage_state["pending"] = None
            # stage W: rolling slots for group g2
            g2 = it - 4
            if 0 <= g2 < NG:
                for q in range(3):
                    emit_slot(3 * g2 + q)
            # stage Z: z matmuls for group g0
            g0 = it
            if g0 < NG:
                for q in range(3):
                    zb = zgp.tile([128, DH], f32, tag="z", name="z")
                    zg_t[(g0, q)] = zb[:]
                    k = 3 * g0 + q
                    j = JD + k
                    jj = j % 4
                    nc.tensor.matmul(zb[:], lhsT(j),
                                     w1[32 * jj:32 * jj + 32, :],
                                     start=True, stop=True,
                                     tile_position=(32 * jj, 0))
        emit_slot(JB)
        emit_slot(JB + 1)
        if stage_state["pending"] is not None:
            ptile, pj0, pgsz = stage_state["pending"]
            nc.sync.dma_start(vb[:, :, pj0:pj0 + pgsz, :], ptile[:, : pgsz * D])
            stage_state["pending"] = None

    nc.compile()
    return nc


def _prep_inputs(inputs):
    f32 = np.float32
    bf = ml_dtypes.bfloat16
    scale = float(np.asarray(inputs["scale"]))

    W1p = np.zeros((32, DH), f32)
    W1p[0:10] = np.asarray(inputs["geom_w1"], f32)
    W1p[6] *= 0.5
    W1p[7] *= 0.5
    W1p -= W1p.mean(axis=1, keepdims=True)      # fold LN mean into W1
    w1rep = np.tile(W1p, (4, 1))

    G = (W1p @ W1p.T).astype(f32)               # gram for LN variance
    Gblk = np.zeros((128, 128), f32)            # block-diag(G x4)
    for t in range(4):
        Gblk[32 * t:32 * t + 32, 32 * t:32 * t + 32] = G

    W2s = scale * np.asarray(inputs["geom_w2"], f32)
    w2hi, w2lo = W2s[:128], W2s[128:]

    cat_t = np.asarray(inputs["cat_table"], f32)
    cam_t = np.asarray(inputs["cam_table"], f32)
    bias_row = (np.asarray(inputs["geom_b2"], f32)
                + np.asarray(inputs["conf_b"], f32)
                + np.asarray(inputs["center_b"], f32))
    w2x_reps = []
    for c in range(3):
        W2x = np.zeros((32, D), f32)
        W2x[6] = scale * np.asarray(inputs["center_w"], f32)[0] * 0.5
        W2x[7] = scale * np.asarray(inputs["center_w"], f32)[1] * 0.5
        W2x[10:13] = scale * cat_t
        W2x[13] = scale * np.asarray(inputs["conf_w"], f32)[0]
        W2x[14] = scale * (bias_row + cam_t[c])
        W2x[15] = np.asarray(inputs["missing_emb"], f32)[0]
        W2x[16] = np.asarray(inputs["dist_w"], f32)[0]
        W2x[17] = np.asarray(inputs["dist_b"], f32)
        w2x_reps.append(np.tile(W2x, (4, 1)))

    idf32 = np.eye(128, dtype=f32)
    bpk = np.concatenate(
        [w1rep, w2hi, w2lo] + w2x_reps + [Gblk, idf32], axis=1
    ).astype(bf)

    box = np.asarray(inputs["box_data"], f32)
    fpks = []
    for c in range(NCORES):
        rawc = box[c * BPC:(c + 1) * BPC].reshape(BPC, T * 6, 6)
        rawc = rawc.reshape(BPC, 8, JB, 6).reshape(128, 900)
        fpks.append(np.ascontiguousarray(
            np.concatenate([rawc, idf32], axis=1), dtype=f32))
    return fpks, bpk


def _fast_path_ok(inputs):
    try:
        shapes = {
            "box_data": (B, T, 6, 6), "cat_table": (3, D), "geom_w1": (10, DH),
            "geom_b1": (DH,), "ln_g": (DH,), "ln_b": (DH,), "geom_w2": (DH, D),
            "geom_b2": (D,), "conf_w": (1, D), "conf_b": (D,),
            "center_w": (2, D), "center_b": (D,), "missing_emb": (1, D),
            "dist_w": (1, D), "dist_b": (D,), "cam_table": (NCAM, D),
        }
        for k, s in shapes.items():
            if tuple(np.asarray(inputs[k]).shape) != s:
                return False
        if not np.all(np.asarray(inputs["geom_b1"]) == 0):
            return False
        if not np.all(np.asarray(inputs["ln_g"]) == 1):
            return False
        if not np.all(np.asarray(inputs["ln_b"]) == 0):
            return False
        return True
    except Exception:
        return False


def _numpy_fallback(inputs):
    # Exact (slow) port of the reference for unexpected inputs.
    import math
    f32 = np.float32
    inp = {k: np.asarray(v) for k, v in inputs.items()}
    coords = inp["box_data"][..., :4].astype(f32)
    category = inp["box_data"][..., 4].astype(np.int32)
    conf = inp["box_data"][..., 5].astype(f32)
    norm = np.array([IW, IH, IW, IH], f32)
    cn = (coords / norm).reshape(B, T, NCAM, NB, 4)
    category = category.reshape(B, T, NCAM, NB)
    conf = conf.reshape(B, T, NCAM, NB, 1)
    presence = (cn.sum(-1) != 0).astype(f32)
    sort_key = category.astype(f32) + (1.0 - presence) * 1000.0
    idx = np.argsort(sort_key, axis=-1, kind="stable")
    cn = np.take_along_axis(cn, idx[..., None], axis=-2)
    category = np.take_along_axis(category, idx, axis=-1)
    conf = np.take_along_axis(conf, idx[..., None], axis=-2)
    presence = (cn.sum(-1) != 0).astype(f32)[..., None]
    x1, y1, x2, y2 = cn[..., 0], cn[..., 1], cn[..., 2], cn[..., 3]
    w, h = x2 - x1, y2 - y1
    cx, cy = (x1 + x2) * 0.5, (y1 + y2) * 0.5
    area, aspect = w * h, w / (h + 1e-6)
    dx, dy = cx[..., 0] - cx[..., 1], cy[..., 0] - cy[..., 1]
    dist = np.sqrt(dx * dx + dy * dy)[..., None]
    dist_tok = dist @ inp["dist_w"].astype(f32) + inp["dist_b"].astype(f32)
    geom = np.stack([x1, y1, x2, y2, w, h, cx, cy, area, aspect], axis=-1)
    z = geom @ inp["geom_w1"].astype(f32) + inp["geom_b1"].astype(f32)
    mu = z.mean(-1, keepdims=True)
    var = ((z - mu) ** 2).mean(-1, keepdims=True)
    xh = (z - mu) / np.sqrt(var + 1e-5) * inp["ln_g"].astype(f32) + inp["ln_b"].astype(f32)
    try:
        from scipy.special import erf as _erf
        g = xh * 0.5 * (1.0 + _erf(xh / np.sqrt(2.0)))
    except Exception:
        verf = np.vectorize(math.erf)
        g = xh * 0.5 * (1.0 + verf(xh / np.sqrt(2.0)))
    geom_p = g @ inp["geom_w2"].astype(f32) + inp["geom_b2"].astype(f32)
    cat_emb = inp["cat_table"].astype(f32)[category]
    conf_p = conf @ inp["conf_w"].astype(f32) + inp["conf_b"].astype(f32)
    center_p = np.stack([cx, cy], axis=-1) @ inp["center_w"].astype(f32) + inp["center_b"].astype(f32)
    cam_emb = inp["cam_table"].astype(f32).reshape(1, 1, NCAM, 1, D)
    tok = (geom_p + cat_emb + conf_p + center_p + cam_emb) * float(inp["scale"])
    tok = np.where(presence == 0, inp["missing_emb"].astype(f32)[0], tok)
    out = np.concatenate([dist_tok.reshape(B, T * NCAM, D),
                          tok.reshape(B, T * NCAM * NB, D)], axis=1)
    return out.astype(np.float32)


def _run(inputs, trace=False, tmpdir=None):
    from concourse.bass_utils import run_bass_kernel_spmd

    if "nc" not in _CACHE:
        _CACHE["nc"] = _build_nc()
    nc = _CACHE["nc"]

    fpks, bpk = _prep_inputs(inputs)
    in_maps = [{"fpk": fpks[c], "bpk": bpk} for c in range(NCORES)]
    res = run_bass_kernel_spmd(nc, in_maps, list(range(NCORES)),
                               trace=trace, tmpdir=tmpdir)
    out = np.concatenate([np.asarray(res.results[c]["out"])
                          for c in range(NCORES)], axis=0)
    return out.astype(np.float32), res


def kernel(**inputs):
    if not _fast_path_ok(inputs):
        return _numpy_fallback(inputs)
    out, _ = _run(inputs)
    return out


if __name__ == "__main__":
    import reference as ref
    inputs = {k: np.asarray(v) for k, v in ref.setup_inputs().items()}
    got = kernel(**inputs)
    exp = np.load("/tmp/expected.npy")
    d = got - exp
    print("rel fro:", np.linalg.norm(d) / np.linalg.norm(exp))
    print("absmax rel:", np.abs(d).max() / np.abs(exp).max())
